# revision 1
# baseline (speedup 1.0000x reference)
"""DCGRU cell Trainium2 kernel: 8-core batch-parallel (B_local=4 per core).

Diffusion (Chebyshev K=2, two supports) via dense-A blocked matmuls
(A shipped [mb, p, kb, m] bf16, streamed from HBM); gate matmuls via
DMA-transposed X^T chunks (round-tripped through DRAM) with zero-padded
per-batch W stationaries chained in PSUM; sigmoid/tanh on ACT with
per-partition bias; PE transposes fold gate outputs back to n-major.
"""
import sys
sys.path.insert(0, "/opt/trn_rl_repo")
import numpy as np
import ml_dtypes

import concourse.bass as bass
import concourse.mybir as mybir
import concourse.tile as tile
import concourse.bacc as bacc
from concourse.bass_utils import run_bass_kernel_spmd
from concourse.masks import make_identity

BF = ml_dtypes.bfloat16
bf16, f32 = mybir.dt.bfloat16, mybir.dt.float32

N, U, D = 8000, 64, 2
B, NCORES = 32, 8
F = D + U
M = 5
BL = B // NCORES
NP = 8064
NW = NP // 128
PK = BL * F
FMT = 384
OC_RU, OC_C = 2 * U, U
NWG = 512
NGRP = (NP + NWG - 1) // NWG
WPG = NWG // 128
AF = mybir.ActivationFunctionType
ALU = mybir.AluOpType


def _combos():
    out = []
    for m in range(M):
        for b_ in range(BL):
            lo, hi = b_ * F, b_ * F + F
            for ch in range(3):
                s, e = max(lo, ch * 128), min(hi, ch * 128 + 128)
                if s < e:
                    out.append((m, ch, b_, s - ch * 128, e - s, s - lo))
    return out


COMBOS = _combos()
CB = {b_: [(i, c[0], c[1]) for i, c in enumerate(COMBOS) if c[2] == b_]
      for b_ in range(BL)}
MCH = sorted({(c[0], c[1]) for c in COMBOS})


def build_program():
    nc = bacc.Bacc()
    x0h_d = nc.declare_dram_parameter("x0h", [128, NW, PK], bf16, isOutput=False)
    A_d = [
        nc.declare_dram_parameter(f"A{s}", [NW, 128, NW, 128], bf16, isOutput=False)
        for s in range(2)
    ]
    wru_d = nc.declare_dram_parameter("Wru", [len(COMBOS), 128, OC_RU], bf16, isOutput=False)
    wc_d = nc.declare_dram_parameter("Wc", [len(COMBOS), 128, OC_C], bf16, isOutput=False)
    bru_d = nc.declare_dram_parameter("bru", [OC_RU, 1], f32, isOutput=False)
    out_d = nc.declare_dram_parameter("out", [BL, NP, U], f32, isOutput=True)

    with tile.TileContext(nc) as tc:
        with (
            tc.tile_pool(name="xpool", bufs=1) as xpool,
            tc.tile_pool(name="apool", bufs=2) as apool,
            tc.tile_pool(name="wres", bufs=1) as wres,
            tc.tile_pool(name="misc", bufs=1) as misc,
            tc.tile_pool(name="xts", bufs=2) as xtsp,
            tc.tile_pool(name="sc", bufs=2) as sc,
            tc.tile_pool(name="dram", bufs=1, space="DRAM") as dram,
            tc.tile_pool(name="dram2", bufs=2, space="DRAM") as dram2,
            tc.tile_pool(name="psA", bufs=3, space="PSUM") as psA,
            tc.tile_pool(name="psW", bufs=2, space="PSUM") as psW,
            tc.tile_pool(name="psT", bufs=2, space="PSUM") as psT,
        ):
            x0 = xpool.tile([128, NW, PK], bf16, tag="x0", name="x0")
            xc = xpool.tile([128, NW, PK], bf16, tag="xc", name="xc")

            bru_t = sc.tile([OC_RU, 1], f32, tag="bru", name="bru")
            nc.sync.dma_start(bru_t[:], bru_d[:])
            ident = sc.tile([128, 128], bf16, tag="ident", name="ident")
            make_identity(nc, ident[:])

            nc.sync.dma_start(x0[:], x0h_d[:])

            HALF = NP // 2

            xm_t = [dram.tile([NP, FMT], bf16, tag=f"xm{m}", name=f"xm{m}")
                    for m in range(M)]
            zpad = sc.tile([128, NW, FMT - PK], bf16, tag="zpad", name="zpad")
            nc.vector.memset(zpad[:], 0.0)
            for m in range(M):
                nc.sync.dma_start(
                    xm_t[m][:, PK:FMT].rearrange("(w p) k -> p w k", p=128),
                    zpad[:],
                )

            def spmm(dst_tile, src_tile, s, scale2, sub_tile, dump_win):
                for mb in range(NW):
                    slab = apool.tile([128, NW, 128], bf16, tag="aslab", name="aslab")
                    nc.sync.dma_start(slab[:], A_d[s][mb])
                    ps = psA.tile([128, PK], f32, tag="ps", name="ps")
                    for kb in range(NW):
                        nc.tensor.matmul(
                            ps[:], slab[:, kb, :], src_tile[:, kb, :],
                            start=(kb == 0), stop=(kb == NW - 1),
                        )
                    if dst_tile is not None:
                        nc.scalar.activation(
                            dst_tile[:, mb, :], ps[:], AF.Copy, scale=float(scale2)
                        )
                    else:
                        stg = xtsp.tile([128, PK], bf16, tag="stg", name="stg")
                        nc.vector.tensor_tensor(
                            out=stg[:], in0=ps[:], in1=sub_tile[:, mb, :],
                            op=ALU.subtract,
                        )
                        dump_win(mb, stg)

            def gconv(w_dram, oc, sig_out):
                def dump_full(src, m):
                    nc.sync.dma_start(
                        xm_t[m][:, 0:PK].rearrange("(w p) k -> p w k", p=128),
                        src[:],
                    )

                dump_full(x0, 0)
                for s in range(2):
                    spmm(xc, x0, s, 2.0, None, None)
                    dump_full(xc, 1 + 2 * s)
                    m2 = 2 + 2 * s

                    def dw(w, stg, m2=m2):
                        nc.sync.dma_start(
                            xm_t[m2][w * 128:(w + 1) * 128, 0:PK], stg[:]
                        )
                    spmm(None, xc, s, 1.0, x0, dw)

                xt_t = dram2.tile([len(MCH), 128, NP], bf16, tag="xt_d", name="xt_d")
                for i, (m, ch) in enumerate(MCH):
                    for h in range(2):
                        xt = misc.tile([128, HALF], bf16, tag="xt", name="xt")
                        nc.sync.dma_start(
                            out=xt[:],
                            in_=xm_t[m][h * HALF:(h + 1) * HALF,
                                        ch * 128:(ch + 1) * 128],
                            transpose=True,
                        )
                        nc.sync.dma_start(
                            xt_t[i][:, h * HALF:(h + 1) * HALF], xt[:]
                        )

                wt = []
                for i in range(len(COMBOS)):
                    t = wres.tile([128, oc], bf16, tag=f"w{i}", name=f"w{i}")
                    nc.sync.dma_start(t[:], w_dram[i])
                    wt.append(t)

                for b_ in range(BL):
                    chain = CB[b_]
                    for g in range(NGRP):
                        lo = g * NWG
                        w_ = min(NWG, NP - lo)
                        pw = psW.tile([oc, NWG], f32, tag="pw", name="pw")
                        for ci, (widx, m, ch) in enumerate(chain):
                            xts = xtsp.tile([128, NWG], bf16, tag="xts", name="xts")
                            nc.sync.dma_start(
                                xts[:, :w_], xt_t[MCH.index((m, ch))][:, lo:lo + w_]
                            )
                            nc.tensor.matmul(
                                pw[:, :w_], wt[widx][:], xts[:, :w_],
                                start=(ci == 0), stop=(ci == len(chain) - 1),
                            )
                        sig_out(b_, g, lo, w_, pw)

            # ------------- gconv 1 (ru) -------------
            u_nd = dram.tile([BL, 128, NW, U], bf16, tag="u_nd", name="u_nd")

            def ru_out(b_, g, lo, w_, pw):
                rsl = xtsp.tile([U, NWG], bf16, tag="rsl", name="rsl")
                nc.scalar.activation(
                    rsl[:, :w_], pw[0:U, :w_], AF.Sigmoid, bias=bru_t[0:U, :]
                )
                usl = xtsp.tile([U, NWG], bf16, tag="usl", name="usl")
                nc.scalar.activation(
                    usl[:, :w_], pw[U:OC_RU, :w_], AF.Sigmoid, bias=bru_t[U:OC_RU, :]
                )
                for j in range(w_ // 128):
                    w = g * WPG + j
                    pt = psT.tile([128, U], bf16, tag="pt", name="pt")
                    nc.tensor.transpose(
                        pt[:], rsl[:, j * 128:(j + 1) * 128], ident[0:U, 0:U]
                    )
                    nc.vector.tensor_tensor(
                        out=x0[:, w, b_ * F + D:(b_ + 1) * F],
                        in0=pt[:],
                        in1=x0[:, w, b_ * F + D:(b_ + 1) * F],
                        op=ALU.mult,
                    )
                    ptu = psT.tile([128, U], bf16, tag="pt", name="ptu")
                    nc.tensor.transpose(
                        ptu[:], usl[:, j * 128:(j + 1) * 128], ident[0:U, 0:U]
                    )
                    ustg = xtsp.tile([128, U], bf16, tag="ustg", name="ustg")
                    nc.vector.tensor_copy(ustg[:], ptu[:])
                    nc.sync.dma_start(u_nd[b_, :, w, :], ustg[:])

            gconv(wru_d, OC_RU, ru_out)

            # ------------- gconv 2 (c) -------------
            c_nd = dram.tile([BL, 128, NW, U], bf16, tag="c_nd", name="c_nd")

            def c_out(b_, g, lo, w_, pw):
                csl = xtsp.tile([U, NWG], bf16, tag="csl", name="csl")
                nc.scalar.activation(csl[:, :w_], pw[:, :w_], AF.Tanh)
                for j in range(w_ // 128):
                    w = g * WPG + j
                    ptc = psT.tile([128, U], bf16, tag="pt", name="ptc")
                    nc.tensor.transpose(
                        ptc[:], csl[:, j * 128:(j + 1) * 128], ident[0:U, 0:U]
                    )
                    cstg = xtsp.tile([128, U], bf16, tag="ustg", name="cstg")
                    nc.vector.tensor_copy(cstg[:], ptc[:])
                    nc.sync.dma_start(c_nd[b_, :, w, :], cstg[:])

            gconv(wc_d, OC_C, c_out)

            # ------------- final combine -------------
            for b_ in range(BL):
                hxs = misc.tile([128, NW, U], bf16, tag="hxs", name="hxs")
                nc.sync.dma_start(hxs[:], x0h_d[:, :, b_ * F + D:(b_ + 1) * F])
                un = misc.tile([128, NW, U], bf16, tag="un", name="un")
                nc.sync.dma_start(un[:], u_nd[b_])
                cn = misc.tile([128, NW, U], bf16, tag="cn", name="cn")
                nc.sync.dma_start(cn[:], c_nd[b_])
                hmc = misc.tile([128, NW, U], bf16, tag="hmc", name="hmc")
                nc.vector.tensor_tensor(out=hmc[:], in0=hxs[:], in1=cn[:],
                                        op=ALU.subtract)
                um = misc.tile([128, NW, U], bf16, tag="um", name="um")
                nc.vector.tensor_tensor(out=um[:], in0=un[:], in1=hmc[:],
                                        op=ALU.mult)
                ost = misc.tile([128, NW, U], bf16, tag="ost", name="ost")
                nc.vector.tensor_tensor(out=ost[:], in0=um[:], in1=cn[:],
                                        op=ALU.add)
                nc.gpsimd.dma_start(
                    out_d[b_].rearrange("(w p) u -> p w u", p=128), ost[:]
                )

    nc.compile()
    return nc


_NC = None


def _host_prep(inputs, hx, row0, col0, val0, row1, col1, val1, W_ru, b_ru, W_c, b_c):
    inp3 = np.asarray(inputs, np.float32).reshape(B, N, D)
    hx3 = np.asarray(hx, np.float32).reshape(B, N, U)

    x0_all = np.zeros((NCORES, 128, NW, PK), BF)
    xf = np.zeros((B, NP, F), np.float32)
    xf[:, :N, :D] = inp3
    xf[:, :N, D:] = hx3
    xfw = xf.reshape(B, NW, 128, F)
    for k_ in range(NCORES):
        for b_ in range(BL):
            x0_all[k_, :, :, b_ * F:(b_ + 1) * F] = (
                xfw[k_ * BL + b_].transpose(1, 0, 2).astype(BF)
            )

    A_blocked = []
    for (r, c, v) in ((row0, col0, val0), (row1, col1, val1)):
        At = np.zeros((NP, NP), np.float32)
        np.add.at(At, (np.asarray(c), np.asarray(r)), np.asarray(v, np.float32))
        Ab = At.reshape(NW, 128, NW, 128).transpose(2, 1, 0, 3)
        A_blocked.append(np.ascontiguousarray(Ab.astype(BF)))

    def build_wzp(Wfull, oc):
        Wm = [np.asarray(Wfull, np.float32)[m::M, :].copy() for m in range(M)]
        Wm[1] *= 0.5
        Wm[3] *= 0.5
        arr = np.zeros((len(COMBOS), 128, oc), np.float32)
        for i, (m, ch, b_, flo, fcnt, foff) in enumerate(COMBOS):
            arr[i, flo:flo + fcnt, :] = Wm[m][foff:foff + fcnt, :]
        return arr.astype(BF)

    return (
        x0_all, A_blocked,
        build_wzp(W_ru, OC_RU), build_wzp(W_c, OC_C),
        np.asarray(b_ru, np.float32).reshape(OC_RU, 1),
    )


def kernel(**inputs):
    global _NC
    if _NC is None:
        _NC = build_program()
    x0_all, A_blocked, wru, wc, bru = _host_prep(**inputs)
    in_maps = [
        {"x0h": x0_all[k_], "A0": A_blocked[0], "A1": A_blocked[1],
         "Wru": wru, "Wc": wc, "bru": bru}
        for k_ in range(NCORES)
    ]
    res = run_bass_kernel_spmd(_NC, in_maps, list(range(NCORES)))
    out = np.zeros((B, N * U), np.float32)
    for k_, r in enumerate(res.results):
        o = np.asarray(r["out"], np.float32)[:, :N, :]
        out[k_ * BL:(k_ + 1) * BL] = o.reshape(BL, N * U)
    return out



# revision 7
# speedup vs baseline: 19.4837x; 19.4837x over previous
"""DCGRU cell Trainium2 kernel: 8-core batch-parallel (B_local=4 per core).

Sparse-graph formulation: edges are shipped as per-block (128 dest rows)
slot tables; SpMM is an indirect-DMA gather of x rows by column index +
a one-hot (iota/is_equal/mult) stationary matmul accumulating y blocks
in PSUM.  All per-core inputs are packed into ONE bf16 blob (one
host->device transfer, ~5.5MB/core vs 266MB dense).  Gate matmuls use
zero-padded per-(m,chunk,batch) W slabs built on device from raw W.
Output is returned transposed [BL, U, NP] bf16 and assembled on host.
"""
import sys
sys.path.insert(0, "/opt/trn_rl_repo")
import numpy as np
import ml_dtypes

import concourse.bass as bass
import concourse.mybir as mybir
import concourse.tile as tile
import concourse.bacc as bacc
from concourse.masks import make_identity

BF = ml_dtypes.bfloat16
bf16, f32, i32 = mybir.dt.bfloat16, mybir.dt.float32, mybir.dt.int32

N, U, D = 8000, 64, 2
B, NCORES = 32, 8
F = D + U            # 66
M = 5
BL = B // NCORES     # 4
PK = BL * F          # 264
NP = 8064            # 63*128
NW = NP // 128       # 63
E = 64000
OC_RU, OC_C = 2 * U, U
GRP = 512
NGRP = (NP + GRP - 1) // GRP     # 16 (last group = 384 cols)
AF = mybir.ActivationFunctionType
ALU = mybir.AluOpType
CPB_DEFAULT = 9


def _combos():
    """Gate-matmul stationaries: per (m, batch) the X^T rows
    [b*F, b*F+F) restricted to one 128-row chunk."""
    out = []
    for m in range(M):
        for b in range(BL):
            lo, hi = b * F, b * F + F
            for ch in range(3):
                s, e = max(lo, ch * 128), min(hi, ch * 128 + 128)
                if s < e:
                    # (m, chunk, plo within chunk, foff within F, count)
                    out.append((m, ch, s - ch * 128, s - lo, e - s))
    return out


COMBOS = _combos()
CB = {b: [] for b in range(BL)}
for i, (m, ch, plo, foff, fcnt) in enumerate(COMBOS):
    b = (ch * 128 + plo - foff) // F
    CB[b].append(i)
MCH = sorted({(c[0], c[1]) for c in COMBOS})

# hx^T row splits per batch: rows [b*F+D, b*F+F) of X^T -> chunk pieces
HX_SPLITS = {}
for b in range(BL):
    r0, r1 = b * F + D, b * F + F
    parts, dst = [], 0
    for ch in range(3):
        s, e = max(r0, ch * 128), min(r1, ch * 128 + 128)
        if s < e:
            parts.append((dst, ch, s - ch * 128, e - s))
            dst += e - s
    HX_SPLITS[b] = parts


def _bcast_inner(ap, inner):
    """Append a stride-0 inner dim of size `inner` to an AP."""
    return bass.AP(ap.tensor, ap.offset, list(ap.ap) + [[0, inner]])


def _blob_layout(cpb):
    ncht = NW * cpb
    off, lay = 0, {}

    def put(name, sz):
        nonlocal off
        lay[name] = (off, sz)
        off += sz

    put("hx", BL * NP * U)
    put("in", BL * NP * D)
    for s in range(2):
        put(f"col{s}", 128 * ncht * 2)   # int32 bitcast as 2x bf16
        put(f"rv{s}", 128 * ncht)
        put(f"val{s}", 128 * ncht)
    put("wru", F * M * OC_RU)
    put("wc", F * M * OC_C)
    put("bru", OC_RU * 2)                # f32 bitcast as 2x bf16
    put("bc", OC_C * 2)
    lay["_total"] = off
    return lay


def build_program(cpb):
    ncht = NW * cpb
    lay = _blob_layout(cpb)
    TOT = lay["_total"]

    nc = bacc.Bacc()
    blob_d = nc.declare_dram_parameter("blob", [TOT], bf16, isOutput=False)
    out_d = nc.declare_dram_parameter("outT", [BL, U, NP], bf16, isOutput=True)

    def bslice(name):
        o, sz = lay[name]
        return blob_d[o:o + sz]

    with tile.TileContext(nc) as tc:
        with (
            tc.tile_pool(name="st", bufs=1) as st,
            tc.tile_pool(name="wk", bufs=3) as wk,
            tc.tile_pool(name="xtw", bufs=2) as xtw,
            tc.tile_pool(name="dram", bufs=1, space="DRAM") as dram,
            tc.tile_pool(name="psA", bufs=2, space="PSUM") as psA,
            tc.tile_pool(name="psT", bufs=2, space="PSUM") as psT,
            tc.tile_pool(name="psW", bufs=2, space="PSUM") as psW,
        ):
            # ---------- static setup ----------
            ident = st.tile([128, 128], bf16, tag="ident", name="ident")
            make_identity(nc, ident[:])

            iota_bf = st.tile([128, cpb * 128], bf16, tag="iota", name="iota")
            nc.gpsimd.iota(iota_bf[:], [[0, cpb], [1, 128]],
                           channel_multiplier=0,
                           allow_small_or_imprecise_dtypes=True)

            cols_t, rv_t, val_t = [], [], []
            for s in range(2):
                ct = st.tile([128, ncht], i32, tag=f"col{s}", name=f"col{s}")
                nc.sync.dma_start(
                    ct[:],
                    bslice(f"col{s}").bitcast(i32).rearrange(
                        "(p n) -> p n", p=128),
                )
                cols_t.append(ct)
                rt = st.tile([128, ncht], bf16, tag=f"rv{s}", name=f"rv{s}")
                nc.sync.dma_start(
                    rt[:], bslice(f"rv{s}").rearrange("(p n) -> p n", p=128))
                rv_t.append(rt)
                vt = st.tile([128, ncht], bf16, tag=f"val{s}", name=f"val{s}")
                nc.sync.dma_start(
                    vt[:], bslice(f"val{s}").rearrange("(p n) -> p n", p=128))
                val_t.append(vt)

            bru_t = st.tile([OC_RU, 1], f32, tag="bru", name="bru")
            nc.sync.dma_start(
                bru_t[:],
                bslice("bru").bitcast(f32).rearrange("(p o) -> p o", p=OC_RU))
            bc_t = st.tile([OC_C, 1], f32, tag="bc", name="bc")
            nc.sync.dma_start(
                bc_t[:],
                bslice("bc").bitcast(f32).rearrange("(p o) -> p o", p=OC_C))

            # W slabs (zero-padded per combo), built on device from raw W
            def build_slabs(wname, oc, tagp):
                wv = bslice(wname).rearrange("(f m o) -> f m o", f=F, m=M)
                slabs = []
                for i, (m, ch, plo, foff, fcnt) in enumerate(COMBOS):
                    t = st.tile([128, oc], bf16, tag=f"{tagp}{i}",
                                name=f"{tagp}{i}")
                    nc.vector.memset(t[:], 0.0)
                    nc.sync.dma_start(t[plo:plo + fcnt, :],
                                      wv[foff:foff + fcnt, m, :])
                    slabs.append(t)
                return slabs

            wru_sl = build_slabs("wru", OC_RU, "wr")
            wc_sl = build_slabs("wc", OC_C, "wc")

            # x0 in SBUF, node-major [128, NW, PK]
            x0_sb = st.tile([128, NW, PK], bf16, tag="x0sb", name="x0sb")
            for b in range(BL):
                o, _ = lay["hx"]
                nc.sync.dma_start(
                    x0_sb[:, :, b * F + D:(b + 1) * F],
                    blob_d[o + b * NP * U: o + (b + 1) * NP * U].rearrange(
                        "(w p u) -> p w u", p=128, u=U),
                )
                o, _ = lay["in"]
                nc.sync.dma_start(
                    x0_sb[:, :, b * F:b * F + D],
                    blob_d[o + b * NP * D: o + (b + 1) * NP * D].rearrange(
                        "(w p u) -> p w u", p=128, u=D),
                )

            # u gate storage (batches 0,1 in rows 0:64/64:128 of ut2[0]...)
            ut2 = [st.tile([128, NP], bf16, tag=f"ut{i}", name=f"ut{i}")
                   for i in range(2)]

            # DRAM x tensors (gather sources) + X^T tensors per gconv
            x0_d = dram.tile([NP, PK], bf16, tag="x0d", name="x0d")
            xa_d = dram.tile([NP, PK], bf16, tag="xad", name="xad")  # A*x0 (scaled 2x)
            xb_d = dram.tile([NP, PK], bf16, tag="xbd", name="xbd")  # cheby2
            xt1 = [dram.tile([3, 128, NP], bf16, tag=f"xt1_{m}",
                             name=f"xt1_{m}") for m in range(M)]
            xt2 = [dram.tile([3, 128, NP], bf16, tag=f"xt2_{m}",
                             name=f"xt2_{m}") for m in range(M)]

            CHW = [128, 128, PK - 256]   # X^T chunk widths

            def emit_xt(src_ap2d, xt_arr_m, mb):
                """PE-transpose a node-major [128, PK] block into X^T dram."""
                for ch in range(3):
                    w = CHW[ch]
                    pt = psT.tile([128, 128], bf16, tag="pt", name="pt")
                    nc.tensor.transpose(
                        pt[0:w, :], src_ap2d[:, ch * 128:ch * 128 + w],
                        ident[:])
                    xts = wk.tile([128, 128], bf16, tag="xts", name="xts")
                    nc.vector.tensor_copy(xts[0:w, :], pt[0:w, :])
                    nc.sync.dma_start(
                        xt_arr_m[ch, 0:w, mb * 128:(mb + 1) * 128],
                        xts[0:w, :])

            def spmm(s, xsrc_d, xdst_d, xt_arr_m, subtract):
                """xdst = 2*(A_s @ xsrc)         (subtract=False)
                   xdst = (A_s @ xsrc) - x0      (subtract=True)
                plus X^T copy of xdst into xt_arr_m."""
                for mb in range(NW):
                    oh = wk.tile([128, cpb * 128], bf16, tag="oh", name="oh")
                    oh3 = oh[:].rearrange("p (c j) -> p c j", c=cpb)
                    nc.vector.tensor_tensor(
                        out=oh3,
                        in0=iota_bf[:].rearrange("p (c j) -> p c j", c=cpb),
                        in1=_bcast_inner(
                            rv_t[s][:, mb * cpb:(mb + 1) * cpb], 128),
                        op=ALU.is_equal,
                    )
                    nc.vector.tensor_tensor(
                        out=oh3, in0=oh3,
                        in1=_bcast_inner(
                            val_t[s][:, mb * cpb:(mb + 1) * cpb], 128),
                        op=ALU.mult,
                    )
                    ps = psA.tile([128, PK], f32, tag="ps", name="ps")
                    for c in range(cpb):
                        g = wk.tile([128, PK], bf16, tag="g", name="g")
                        gci = mb * cpb + c
                        nc.gpsimd.indirect_dma_start(
                            out=g[:], out_offset=None,
                            in_=xsrc_d[:],
                            in_offset=bass.IndirectOffsetOnAxis(
                                ap=cols_t[s][:, gci:gci + 1], axis=0),
                        )
                        nc.tensor.matmul(
                            ps[:], oh[:, c * 128:(c + 1) * 128], g[:],
                            start=(c == 0), stop=(c == cpb - 1))
                    stage = wk.tile([128, PK], bf16, tag="stage", name="stage")
                    if subtract:
                        nc.vector.tensor_tensor(
                            out=stage[:], in0=ps[:], in1=x0_sb[:, mb, :],
                            op=ALU.subtract)
                    else:
                        nc.scalar.activation(stage[:], ps[:], AF.Copy,
                                             scale=2.0)
                    nc.sync.dma_start(
                        xdst_d[mb * 128:(mb + 1) * 128, :], stage[:])
                    emit_xt(stage[:], xt_arr_m, mb)

            def gconv(x0src_d, xt_arr, slabs, oc, out_fn):
                # X^T of x0
                for mb in range(NW):
                    emit_xt(x0_sb[:, mb, :], xt_arr[0], mb)
                for s in range(2):
                    spmm(s, x0src_d, xa_d, xt_arr[1 + 2 * s], subtract=False)
                    spmm(s, xa_d, xb_d, xt_arr[2 + 2 * s], subtract=True)
                # gates
                for gidx in range(NGRP):
                    lo = gidx * GRP
                    w = min(GRP, NP - lo)
                    xtg = {}
                    for (m, ch) in MCH:
                        t = xtw.tile([128, GRP], bf16, tag=f"xt{m}_{ch}",
                                     name=f"xtg{m}_{ch}")
                        hc = CHW[ch]   # only rows actually written in DRAM
                        nc.sync.dma_start(t[0:hc, :w],
                                          xt_arr[m][ch, 0:hc, lo:lo + w])
                        xtg[(m, ch)] = t
                    for b in range(BL):
                        pw = psW.tile([128, GRP], f32, tag="pw", name="pw")
                        chain = CB[b]
                        for ci, si in enumerate(chain):
                            m, ch = COMBOS[si][0], COMBOS[si][1]
                            hc = CHW[ch]   # clamp K to rows written in DRAM
                            nc.tensor.matmul(
                                pw[0:oc, :w],
                                slabs[si][0:hc, :],
                                xtg[(m, ch)][0:hc, :w],
                                start=(ci == 0), stop=(ci == len(chain) - 1))
                        out_fn(b, lo, w, pw)

            # ---------- gconv 1: r/u gates ----------
            nc.sync.dma_start(
                x0_d[:].rearrange("(w p) k -> p w k", p=128), x0_sb[:])

            def ru_out(b, lo, w, pw):
                rt = wk.tile([U, GRP], bf16, tag="rt", name="rt")
                nc.scalar.activation(rt[:, :w], pw[0:U, :w], AF.Sigmoid,
                                     bias=bru_t[0:U, :])
                nc.scalar.activation(
                    ut2[b // 2][(b % 2) * U:(b % 2 + 1) * U, lo:lo + w],
                    pw[U:2 * U, :w], AF.Sigmoid, bias=bru_t[U:2 * U, :])
                for j in range(w // 128):
                    wblk = lo // 128 + j
                    rp = psT.tile([128, U], bf16, tag="rp", name="rp")
                    nc.tensor.transpose(
                        rp[:], rt[:, j * 128:(j + 1) * 128], ident[0:U, 0:U])
                    nc.vector.tensor_tensor(
                        out=x0_sb[:, wblk, b * F + D:(b + 1) * F],
                        in0=rp[:],
                        in1=x0_sb[:, wblk, b * F + D:(b + 1) * F],
                        op=ALU.mult)

            gconv(x0_d, xt1, wru_sl, OC_RU, ru_out)

            # ---------- gconv 2: c gate ----------
            x0b_d = dram.tile([NP, PK], bf16, tag="x0bd", name="x0bd")
            nc.sync.dma_start(
                x0b_d[:].rearrange("(w p) k -> p w k", p=128), x0_sb[:])

            def c_out(b, lo, w, pw):
                # all SBUF operands share base partition q so DVE
                # tensor_tensor base-partition constraints hold
                q = (b % 2) * U
                ct = wk.tile([128, GRP], bf16, tag="ct", name="ct")
                nc.scalar.activation(ct[q:q + U, :w], pw[0:U, :w], AF.Tanh,
                                     bias=bc_t[:])
                hxg = wk.tile([128, GRP], bf16, tag="hxg", name="hxg")
                for (dst0, ch, p0, cnt) in HX_SPLITS[b]:
                    nc.sync.dma_start(hxg[q + dst0:q + dst0 + cnt, :w],
                                      xt1[0][ch, p0:p0 + cnt, lo:lo + w])
                dt_ = wk.tile([128, GRP], bf16, tag="dt", name="dt")
                nc.vector.tensor_tensor(out=dt_[q:q + U, :w],
                                        in0=hxg[q:q + U, :w],
                                        in1=ct[q:q + U, :w], op=ALU.subtract)
                ot = wk.tile([128, GRP], bf16, tag="ot", name="ot")
                nc.vector.tensor_tensor(
                    out=ot[q:q + U, :w],
                    in0=ut2[b // 2][q:q + U, lo:lo + w],
                    in1=dt_[q:q + U, :w], op=ALU.mult)
                nc.vector.tensor_tensor(out=ot[q:q + U, :w],
                                        in0=ot[q:q + U, :w],
                                        in1=ct[q:q + U, :w], op=ALU.add)
                nc.sync.dma_start(out_d[b][:, lo:lo + w], ot[q:q + U, :w])

            gconv(x0b_d, xt2, wc_sl, OC_C, c_out)

    nc.compile()
    return nc


# ---------------------------------------------------------------------------
# host side
# ---------------------------------------------------------------------------

def _prep_tables(row, col, val, cpb):
    r = np.asarray(row)
    c = np.asarray(col)
    v = np.asarray(val, np.float32)
    blk = r >> 7
    order = np.argsort(blk, kind="stable")
    rs, cs, vs = r[order], c[order], v[order]
    blks = blk[order]
    cnt = np.bincount(blks, minlength=NW)
    need = -(-int(cnt.max()) // 128)
    starts = np.concatenate([[0], np.cumsum(cnt)[:-1]])
    pos = np.arange(E) - np.repeat(starts, cnt)
    slots = blks * (cpb * 128) + pos
    colT = np.zeros(NW * cpb * 128, np.int32)
    colT[slots] = cs
    rvT = np.zeros(NW * cpb * 128, np.float32)
    rvT[slots] = rs & 127
    vT = np.zeros(NW * cpb * 128, np.float32)
    vT[slots] = vs
    colT = np.ascontiguousarray(colT.reshape(-1, 128).T)
    rvT = np.ascontiguousarray(rvT.reshape(-1, 128).T).astype(BF)
    vT = np.ascontiguousarray(vT.reshape(-1, 128).T).astype(BF)
    return colT, rvT, vT, need


def _edges_cpb(row0, row1):
    need = 1
    for r in (np.asarray(row0), np.asarray(row1)):
        cnt = np.bincount(r >> 7, minlength=NW)
        need = max(need, -(-int(cnt.max()) // 128))
    return need


def _prep_w(W, oc):
    Wf = np.array(W, np.float32)
    m = np.arange(Wf.shape[0]) % M
    Wf[(m == 1) | (m == 3)] *= 0.5
    return Wf.astype(BF)


def _host_prep(cpb, inputs, hx, row0, col0, val0, row1, col1, val1,
               W_ru, b_ru, W_c, b_c):
    lay = _blob_layout(cpb)
    blob = np.zeros((NCORES, lay["_total"]), BF)

    o, sz = lay["hx"]
    bv = blob[:, o:o + sz].reshape(NCORES, BL, NP, U)
    bv[:, :, :N, :] = np.asarray(hx, np.float32).reshape(
        NCORES, BL, N, U).astype(BF)
    o, sz = lay["in"]
    bv = blob[:, o:o + sz].reshape(NCORES, BL, NP, D)
    bv[:, :, :N, :] = np.asarray(inputs, np.float32).reshape(
        NCORES, BL, N, D).astype(BF)

    for s, (r, c, v) in enumerate(((row0, col0, val0), (row1, col1, val1))):
        colT, rvT, vT, need = _prep_tables(r, c, v, cpb)
        assert need <= cpb
        o, sz = lay[f"col{s}"]
        blob[:, o:o + sz] = colT.view(np.uint16).view(BF).reshape(-1)[None]
        o, sz = lay[f"rv{s}"]
        blob[:, o:o + sz] = rvT.reshape(-1)[None]
        o, sz = lay[f"val{s}"]
        blob[:, o:o + sz] = vT.reshape(-1)[None]

    o, sz = lay["wru"]
    blob[:, o:o + sz] = _prep_w(W_ru, OC_RU).reshape(-1)[None]
    o, sz = lay["wc"]
    blob[:, o:o + sz] = _prep_w(W_c, OC_C).reshape(-1)[None]
    o, sz = lay["bru"]
    blob[:, o:o + sz] = np.asarray(b_ru, np.float32).view(
        np.uint16).view(BF).reshape(-1)[None]
    o, sz = lay["bc"]
    blob[:, o:o + sz] = np.asarray(b_c, np.float32).view(
        np.uint16).view(BF).reshape(-1)[None]
    return blob


_CACHE = {}


def _get_exec(cpb):
    key = ("exec", cpb)
    if key in _CACHE:
        return _CACHE[key]

    import jax
    import jax.numpy as jnp
    from jax.sharding import Mesh, PartitionSpec, NamedSharding
    import warnings
    with warnings.catch_warnings():
        warnings.simplefilter("ignore")
        from jax.experimental.shard_map import shard_map
    from concourse import bass2jax

    nc = build_program(cpb)
    bass2jax.install_neuronx_cc_hook()

    partition_name = (nc.partition_id_tensor.name
                      if nc.partition_id_tensor else None)
    in_names, out_names, out_avals, zero_shapes = [], [], [], []
    for alloc in nc.m.functions[0].allocations:
        if not isinstance(alloc, mybir.MemoryLocationSet):
            continue
        name = alloc.memorylocations[0].name
        if alloc.kind == "ExternalInput":
            if name != partition_name:
                in_names.append(name)
        elif alloc.kind == "ExternalOutput":
            shape = tuple(alloc.tensor_shape)
            dtype = mybir.dt.np(alloc.dtype)
            out_names.append(name)
            out_avals.append(jax.core.ShapedArray(shape, dtype))
            zero_shapes.append((shape, dtype))

    dbg_input = None
    if nc.dbg_addr is not None:
        assert not nc.dbg_callbacks
        dbg_input = nc.dbg_addr.name

    n_params = len(in_names)
    n_outs = len(out_names)
    all_in = list(in_names) + list(out_names)
    if partition_name is not None:
        all_in.append(partition_name)
    donate = tuple(range(n_params, n_params + n_outs))

    devices = jax.devices()[:NCORES]
    mesh = Mesh(np.asarray(devices), ("core",))
    P = PartitionSpec

    def _body(*args):
        operands = list(args)
        if partition_name is not None:
            operands.append(bass2jax.partition_id_tensor())
        outs = bass2jax._bass_exec_p.bind(
            *operands,
            out_avals=tuple(out_avals),
            in_names=tuple(all_in),
            out_names=tuple(out_names),
            lowering_input_output_aliases=(),
            sim_require_finite=True,
            sim_require_nnan=True,
            nc=nc,
        )
        return tuple(outs)

    sharded = jax.jit(
        shard_map(_body, mesh=mesh,
                  in_specs=(P("core"),) * (n_params + n_outs),
                  out_specs=(P("core"),) * n_outs, check_rep=False),
        donate_argnums=donate, keep_unused=True)

    def zfn_builder(shape, dtype):
        glob = (NCORES * shape[0],) + tuple(shape[1:])
        return jax.jit(lambda: jnp.zeros(glob, dtype),
                       out_shardings=NamedSharding(mesh, P("core")))

    zfns = [zfn_builder(s, d) for (s, d) in zero_shapes]

    ex = {
        "sharded": sharded, "zfns": zfns, "in_names": in_names,
        "out_names": out_names, "dbg": dbg_input, "nc": nc,
    }
    _CACHE[key] = ex
    return ex


def kernel(**inputs):
    cpb = max(CPB_DEFAULT, _edges_cpb(inputs["row0"], inputs["row1"]))
    ex = _get_exec(cpb)
    blob = _host_prep(cpb, **inputs)

    arrs = {"blob": np.ascontiguousarray(blob.reshape(-1))}
    if ex["dbg"] is not None:
        arrs[ex["dbg"]] = np.zeros((NCORES, 2), np.uint32)
    args = [arrs[name] for name in ex["in_names"]]
    args += [zf() for zf in ex["zfns"]]
    outs = ex["sharded"](*args)

    o = np.asarray(outs[0]).reshape(NCORES, BL, U, NP)[:, :, :, :N]
    out = o.transpose(0, 1, 3, 2).astype(np.float32).reshape(B, N * U)
    return out
